# revision 46
# baseline (speedup 1.0000x reference)
"""Multi-head attention block (B=2, N=2048, C=1024, H=16, D=64) on 8 TRN2
NeuronCores.

Sharding: tensor-parallel over heads - 2 heads per core, both batch elements.
Each core computes qkv for its 2 heads, full attention for its 4 (batch, head)
pairs, and a partial output projection over its 128 columns of the attention
output. The host sums the 8 fp16 partial projections and adds the bias.

v3 design (vs the 300us baseline):
  - S matmuls row-tiled across the head pair: kT/qT keep head0 on partitions
    0-63 and head1 on 64-127, so the two K=64 S matmuls land on disjoint PE
    row-groups (tile_position (0,0)/(64,0) auto-derived from base partitions)
    and run concurrently - S cost halves.
  - j-major attention steps: each step computes ST for 512 queries x 128 keys
    for BOTH heads into one [128,1024] PSUM pair tile, one [128,1024] exp on
    ACT (the overall bottleneck: 128 exps ~= 147us), then two M=65 PV matmuls
    (V' carries a ones column so the softmax denominator accumulates free).
    PV lags one step behind exp so the in-order PE queue never waits.
  - V' is computed directly token-major (x chunk as stationary) - no PE
    transposes at all.
  - PSUM budget exactly 8 banks: ST pair [128,1024]x2 + OT [65,512]x2 +
    filler [128,512]x2 (qkv/V'/proj/warmup share the filler pool).
  - HAM clock gate: warm-up matmuls at t=0, and a credit-based fill queue
    paces qkv/proj work into the per-step PE slack so the PE never idles
    long enough to re-throttle to K=4/8.
  - Normalization: DVE reciprocal in 4x[1,128] chunks paced through the
    fill queue at cost 300 each (cost-0 units drained all at once and
    monopolized DVE ~6us at half boundaries: PSUM-eviction CASTs queued
    behind them and the in-order PE FIFO stalled on the PSUM WAR - the
    dominant source of exp gaps). NOT reciprocal_approx_fast (passes
    CoreSim, NaNs on hardware); NOT gpsimd.tensor_mul (measured 385us).
    partition_broadcast on GpSimd, multiply on DVE.
  - Startup: only wqkv+xt4 DMA at t=0 (transfers fair-share HBM BW no
    matter the queue/order, so priority = deferred issue); xt5/xt6 are
    chained via GpSimd gate-reads (RAW on prev chunk -> WAR on next
    buffer); later chunks stagger off the xt pool's bufs=4 WAR chain
    (longer gate chains block the norm partition_broadcasts). xT is
    staged host-side chunk-major so each chunk DMA is one contiguous
    8KB/partition read. 30 warm-up matmuls bridge the whole ~10us DMA
    shadow (any >3.4us PE idle re-throttles the HAM clock to 1.2GHz).
  - proj units spread one rb per 4 steps; tail norm splits h0 on DVE /
    h1 on ACT (Ln+Exp) plus keep-warm matmuls during the norm window.
  - y partials in fp16 (halves output DMA); host sums in f32.

Measured: 232-236us (official v3 baseline: 256.5us); rel err 7.0e-04.
Engine busy: PE ~192us (the binding constraint), ACT exp 128x1.1=140us,
DVE ~135us. Ideas that did NOT work: fp8 (tolerance math fails: ~3-6%
value noise vs 2e-2 budget), fp16 PSUM matmul outputs (TRN3-only),
paired [128,2048] exps (PSUM bank budget), DMA priority via queues /
issue order / queue-gate DMAs (transfers always fair-share HBM),
gpsimd tensor_mul for the norm (6x slower than DVE end-to-end).
"""
import sys

sys.path.insert(0, "/opt/trn_rl_repo")

import numpy as np

B = 2
N = 2048
C = 1024
H = 16
D = 64
R = B * N            # 4096 flattened rows
NCORES = 8
HPC = H // NCORES    # heads per core = 2
SCALE = 1.0 / np.sqrt(D)  # 0.125

_NC_CACHE = None


def build_nc():
    import concourse.bass as bass
    import concourse.tile as tile
    from concourse import bacc, mybir

    F32 = mybir.dt.float32
    FP16 = mybir.dt.float16
    Exp = mybir.ActivationFunctionType.Exp

    nc = bacc.Bacc("TRN2", target_bir_lowering=False, debug=False,
                   num_devices=NCORES)

    # xT is staged host-side as [rb, p, a, r] (chunk-major, partition-
    # contiguous) so each chunk DMA is a single contiguous 8KB/partition
    # read; the previous "(a p) r -> p a r" gather ran at ~55% of HBM BW
    # (1KB lines) and made the startup DMA-latency-bound.
    xT_d = nc.declare_dram_parameter("xT", [R // 512, 128, C // 128, 512],
                                     FP16, isOutput=False)
    wqkvT_d = nc.declare_dram_parameter("wqkvT", [128, C // 128, 3 * 2 * D],
                                        FP16, isOutput=False)
    wprojT_d = nc.declare_dram_parameter("wprojT", [2 * D, C], FP16,
                                         isOutput=False)
    y_d = nc.declare_dram_parameter("y", [R, C], FP16, isOutput=True)

    CC = C // 128    # 8 contraction chunks
    NMC = N // 128   # 16 key chunks per batch

    with tile.TileContext(nc) as tc:
        with (
            tc.tile_pool(name="const", bufs=1) as const,
            tc.tile_pool(name="qkvT", bufs=1) as qkvp,
            tc.tile_pool(name="vprime", bufs=1) as vpp,
            tc.tile_pool(name="otbuf", bufs=1) as otp,
            tc.tile_pool(name="xt", bufs=4) as xtp,
            tc.tile_pool(name="et", bufs=3) as etp,
            tc.tile_pool(name="small", bufs=4) as small,
            tc.tile_pool(name="ysb", bufs=4) as ysbp,
            tc.tile_pool(name="stp", bufs=2, space="PSUM") as stp,
            tc.tile_pool(name="fillp", bufs=2, space="PSUM") as fillp,
            tc.tile_pool(name="otps", bufs=1, space="PSUM") as otps,
        ):
            # ---- constants ----
            wqkv_sb = const.tile([128, CC, 3 * 2 * D], FP16)
            wproj_sb = const.tile([128, C], FP16)
            warm = const.tile([128, 512], FP16)

            # ---- persistent activations ----
            qT = qkvp.tile([128, R], FP16)   # rows: h0 d-major | h1 d-major
            kT = qkvp.tile([128, R], FP16)
            vprime = [[vpp.tile([128, NMC, D + 1], FP16, tag=f"vp{b}{hl}",
                                name=f"vp{b}{hl}")
                       for hl in range(HPC)] for b in range(B)]
            # normalized attn out, c-major; one tile per (b, qh, j) 512-token
            # block so the tile-granular dep tracker never makes a proj wait
            # on an unrelated block's norm writes
            ot = {(b, qh, j): otp.tile([128, 512], FP16,
                                       tag=f"ot{b}{qh}{j}",
                                       name=f"ot{b}{qh}{j}")
                  for b in range(B) for qh in range(2) for j in range(2)}

            for b in range(B):
                for hl in range(HPC):
                    nc.gpsimd.memset(vprime[b][hl][:, :, D:D + 1], 1.0)

            # ---- building blocks ----
            xts = {}

            def xt_load(rb, eng=None, tile_=None):
                xt = tile_ if tile_ is not None else xtp.tile(
                    [128, CC, 512], FP16, tag="xt", name="xt")
                (eng or nc.sync).dma_start(xt[:], xT_d[rb])
                xts[rb] = xt

            def qk_half(rb, ob, lo, state):
                # half of a q/k chain (4 contraction chunks, ~1us of PE) so
                # fill units never delay the S/exp cadence by more than that
                col0 = rb * 512
                dst = (qT, kT)[ob]
                if lo == 0:
                    state["ps"] = fillp.tile([128, 512], F32, tag="fill",
                                             name="qkps")
                ps = state["ps"]
                for cc in range(lo, lo + CC // 2):
                    nc.tensor.matmul(
                        ps[:],
                        wqkv_sb[:, cc, ob * 128:(ob + 1) * 128],
                        xts[rb][:, cc, :],
                        start=(cc == 0), stop=(cc == CC - 1),
                    )
                if lo:
                    nc.vector.tensor_copy(dst[:, col0:col0 + 512], ps[:])

            def qk_group(rb, ob):
                st = {}
                qk_half(rb, ob, 0, st)
                qk_half(rb, ob, CC // 2, st)

            def vprime_chunk(b, mc):
                # V' for one 128-token chunk, token-major, both heads at once
                rb = (b * N + mc * 128) // 512
                tok0 = (b * N + mc * 128) % 512
                ps = fillp.tile([128, 512], F32, tag="fill", name="vpps")
                for cc in range(CC):
                    nc.tensor.matmul(
                        ps[:, 0:128],
                        xts[rb][:, cc, tok0:tok0 + 128],
                        wqkv_sb[:, cc, 2 * 128:3 * 128],
                        start=(cc == 0), stop=(cc == CC - 1),
                    )
                for hl in range(HPC):
                    nc.vector.tensor_copy(
                        vprime[b][hl][:, mc, 0:D],
                        ps[:, hl * D:(hl + 1) * D])

            otus = {}

            def evict_ot(b, qh, j, hl, ot_ps):
                otu = small.tile([D + 1, 512], F32, tag=f"otu{hl}",
                                 name="otu")
                nc.vector.tensor_copy(otu[:], ot_ps[:])
                otus[(b, qh, j, hl)] = otu

            def norm_unit(b, qh, j, hl):
                # chunked reciprocal + final mul on DVE, partition broadcast
                # on GpSimd (reciprocal_approx_fast passes CoreSim but NaNs
                # on hardware through this compile path - do not use)
                p0 = hl * D

                def _recip(ch):
                    if ch == 0:
                        rinv = small.tile([1, 512], F32, tag="rinv",
                                          name="rinv")
                        otus[(b, qh, j, hl)] = (otus[(b, qh, j, hl)], rinv)
                    otu, rinv = otus[(b, qh, j, hl)]
                    nc.vector.reciprocal(
                        rinv[:, ch * 128:(ch + 1) * 128],
                        otu[D:D + 1, ch * 128:(ch + 1) * 128])

                def _mul():
                    otu, rinv = otus.pop((b, qh, j, hl))
                    rbig = small.tile([D, 512], F32, tag="rbig", name="rbig")
                    nc.gpsimd.partition_broadcast(rbig[:], rinv[:])
                    nc.vector.tensor_mul(
                        ot[(b, qh, j)][p0:p0 + D, :], otu[0:D, :], rbig[:])

                return [lambda ch=ch: _recip(ch) for ch in range(4)] + [_mul]

            ysbs = {}

            def proj_unit(rb, j, eng="v"):
                # partial y for one 128-token block, 512 output cols
                ps = fillp.tile([128, 512], F32, tag="fill", name="yp")
                src = ot[(rb // 16, (rb // 8) % 2, (rb // 4) % 2)]
                col0 = (rb % 4) * 128
                nc.tensor.matmul(
                    ps[:],
                    src[:, col0:col0 + 128],
                    wproj_sb[:, j * 512:(j + 1) * 512],
                    start=True, stop=True,
                )
                if rb not in ysbs:
                    ysbs[rb] = ysbp.tile([128, C], FP16, tag="ysb",
                                         name="ysb")
                ysb = ysbs[rb]
                if eng == "v":
                    nc.vector.tensor_copy(ysb[:, j * 512:(j + 1) * 512],
                                          ps[:])
                else:
                    nc.scalar.copy(ysb[:, j * 512:(j + 1) * 512], ps[:])
                if j == 1:
                    nc.sync.dma_start(y_d[rb * 128:(rb + 1) * 128, :],
                                      ysb[:])
                    del ysbs[rb]

            # ---- fill queue: paces PE-filler work into per-step slack.
            # Emission order defines RAW deps, so consumers force-drain the
            # queue up to their producer's key before emitting (need()).
            class FillQueue:
                def __init__(self):
                    self.units = []   # (cost_ns, fn, key)
                    self.i = 0
                    self.credit = 2600.0
                    self.cap = 4000.0
                    self.done = set()

                def add(self, cost, fn, key=None):
                    self.units.append((cost, fn, key))

                def _run(self):
                    cost, fn, key = self.units[self.i]
                    fn()
                    if key is not None:
                        self.done.add(key)
                    self.i += 1
                    return cost

                def step(self, slack):
                    self.credit = min(self.credit + slack, self.cap)
                    while self.i < len(self.units):
                        if self.units[self.i][0] > self.credit:
                            break
                        self.credit -= self._run()

                def need(self, key):
                    if key in self.done:
                        return
                    assert any(u[2] == key for u in self.units[self.i:]), key
                    while key not in self.done:
                        self._run()

                def drain(self):
                    while self.i < len(self.units):
                        self._run()

            fq = FillQueue()

            # ---- attention pipeline ----
            pend = [None]   # PV one step behind exp, carried across halves

            def flush_pend():
                if pend[0] is None:
                    return
                b_, qh_, j_, mc_, et_, ops_ = pend[0]
                pend[0] = None
                fq.need(("v", b_, mc_))
                for hl in range(HPC):
                    nc.tensor.matmul(
                        ops_[hl][:],
                        vprime[b_][hl][:, mc_, :],
                        et_[:, hl * 512:(hl + 1) * 512],
                        start=(mc_ == 0), stop=(mc_ == NMC - 1),
                    )
                if mc_ == NMC - 1:
                    for hl in range(HPC):
                        evict_ot(b_, qh_, j_, hl, ops_[hl])
                    # norm work for this j becomes available now; it runs on
                    # DVE/GpSimd during the next ~16 steps, well before any
                    # proj unit for these tokens reaches the PE queue. The
                    # final block's norm instead runs on ACT in the tail.
                    if (b_, qh_, j_) != (0, 1, 1):
                        # modest credit costs: cost-0 norm units all
                        # drained in one step, monopolizing DVE for ~6us
                        # at half boundaries; eviction CASTs queued behind
                        # them and the in-order PE FIFO stalled on the
                        # PSUM WAR. Small per-unit costs spread them a
                        # step apart WITHOUT oversubscribing the pacer
                        # (1650-credit units starved the b=0 qk fill).
                        for hl in range(HPC):
                            for u in norm_unit(b_, qh_, j_, hl):
                                fq.add(300, u)

            def add_proj(rbs):
                for rb in rbs:
                    for j in range(2):
                        fq.add(300, lambda rb=rb, j=j: proj_unit(rb, j))

            def attention_half(b, qh, slack=510.0, proj_rbs=()):
                # proj units are injected one rb every 4 steps (bursts of 8
                # starved the b=0 qk fill and caused multi-us exp gaps at
                # half boundaries); each lands after its block's norm units
                # in the fill list
                flush_pend()
                q0 = b * N + qh * 1024
                for j in range(2):
                    ot_ps = [otps.tile([D + 1, 512], F32, tag=f"ot{hl}",
                                       name=f"otps{hl}")
                             for hl in range(HPC)]
                    qcol = q0 + j * 512
                    fq.need(("q", qcol // 512))
                    for mc in range(NMC):
                        idx = j * NMC + mc
                        if idx % 4 == 2 and idx // 4 < len(proj_rbs):
                            add_proj([proj_rbs[idx // 4]])
                        kcol = b * N + mc * 128
                        fq.need(("k", kcol // 512))
                        st = stp.tile([128, 1024], F32, tag="stp",
                                      name="st")
                        for hl in range(HPC):
                            nc.tensor.matmul(
                                st[:, hl * 512:(hl + 1) * 512],
                                kT[hl * D:(hl + 1) * D, kcol:kcol + 128],
                                qT[hl * D:(hl + 1) * D, qcol:qcol + 512],
                                start=True, stop=True,
                            )
                        et = etp.tile([128, 1024], FP16, tag="et", name="et")
                        nc.scalar.activation(et[:], st[:], Exp, scale=SCALE)
                        # fill AFTER the S pair: the paced units land in the
                        # PE queue between S(t) and PV(t-1), absorbing the
                        # window where PV would stall on exp(t-1)'s sem
                        fq.step(slack)
                        flush_pend()
                        pend[0] = (b, qh, j, mc, et, ot_ps)

            # ---- emission ----
            with nc.named_scope("startup"):
                # DMAs fan out across four engine queues so the transfers
                # overlap: wqkv (first dependency) on sync, xt4 (second) on
                # scalar, the rest spread out. The warm-tile memset runs on
                # DVE (GpSimd takes ~5us to boot its library - gating the
                # HAM warm-up on it costs 7us of dead time at t=0).
                nc.vector.memset(warm[:], 0.125)
                # DMA priority: transfers issued from one engine queue run
                # one-at-a-time, so each queue is an ordered priority lane;
                # issuing everything at once instead round-robins the rings
                # and starves the first-needed transfer (wqkv landed at 18us
                # and the whole pipeline started cold).
                # DMA transfers all fair-share HBM bandwidth concurrently
                # regardless of issue queue or order, so priority requires
                # DEFERRED ISSUE via data deps. At t=0 only wqkv+xt4 move
                # (the first chain's deps, 1.8MB -> land ~5us after ctx
                # start). Every later chunk is chained through a gate on
                # the idle GpSimd queue: copy(prev chunk) RAW-blocks until
                # the previous transfer lands, then copy(this buffer)
                # creates the WAR that holds this chunk's DMA until then.
                # Result: strictly serialized transfers, each at full HBM
                # bandwidth, in need order.
                nc.sync.dma_start(wqkv_sb[:], wqkvT_d[:])
                xt_load(4)
                nc.scalar.dma_start(wproj_sb[:], wprojT_d[:])
                # gate only xt5/xt6: later chunks are staggered by the
                # xt pool's bufs=4 WAR chain anyway, and a longer GpSimd
                # gate chain blocks the first norm partition_broadcast
                # behind xt3's landing (~55us) - the old step-23 exp gap
                prev = wqkv_sb[0:1, 0, 0:16]
                for rb in (5, 6, 7, 0, 1, 2, 3):
                    xt = xtp.tile([128, CC, 512], FP16, tag="xt", name="xt")
                    if rb in (5, 6):
                        nc.gpsimd.memset(xt[0:1, 0, 0:16], 0.0)
                        gsc = small.tile([1, 16], FP16, tag="gsc",
                                         name="gsc")
                        nc.gpsimd.tensor_copy(gsc[:], prev)
                        nc.gpsimd.tensor_copy(gsc[:], xt[0:1, 0, 0:16])
                    xt_load(rb, tile_=xt)
                    prev = xt[0:1, 0, 0:16]
                # HAM warm-up: back-to-back matmuls on a memset tile lift
                # the PE clock gate to K=8/8 while the DMAs land, so the
                # first qkv chains run at 2.4 GHz (few enough that they
                # don't delay the first qk chain behind them in the queue)
                for w in range(30):
                    ps = fillp.tile([128, 512], F32, tag="fill", name="warm")
                    nc.tensor.matmul(ps[:], warm[:, 0:128], warm[:],
                                     start=True, stop=True)
                # minimum work for the first attention half (b=1, qh=0)
                qk_group(4, 1)
                qk_group(4, 0)
            fq.done.update([("k", 4), ("q", 4)])

            # global ordered fill list; hw deps gate execution, the queue
            # only paces emission into PE slack. Order follows need time in
            # the attn10 pipeline; need() force-drains stragglers.
            def add_qk(rb, ob, key):
                st = {}
                fq.add(1000, lambda: qk_half(rb, ob, 0, st))
                fq.add(1000, lambda: qk_half(rb, ob, CC // 2, st), key)

            add_qk(5, 1, ("k", 5))
            for mc in range(8):
                fq.add(600, lambda mc=mc: vprime_chunk(1, mc), ("v", 1, mc))
            add_qk(6, 1, ("k", 6))
            add_qk(7, 1, ("k", 7))
            for mc in range(8, 16):
                fq.add(600, lambda mc=mc: vprime_chunk(1, mc), ("v", 1, mc))
            add_qk(5, 0, ("q", 5))
            add_qk(6, 0, ("q", 6))
            add_qk(7, 0, ("q", 7))
            for rb in range(4):
                add_qk(rb, 1, ("k", rb))
                for mc in range(4 * rb, 4 * rb + 4):
                    fq.add(600, lambda mc=mc: vprime_chunk(0, mc),
                           ("v", 0, mc))
                add_qk(rb, 0, ("q", rb))

            with nc.named_scope("attn10"):
                attention_half(1, 0, slack=800.0)
            with nc.named_scope("attn11"):
                attention_half(1, 1, proj_rbs=range(16, 24))
            with nc.named_scope("attn00"):
                attention_half(0, 0, proj_rbs=range(24, 32))
            with nc.named_scope("attn01"):
                attention_half(0, 1, proj_rbs=range(0, 8))

            with nc.named_scope("tail"):
                fq.drain()
                # rb 8-11 only need norm(0,1,j0) (ran mid-attn01); keep the
                # PE busy on them while the last exp + PV finish
                for rb in range(8, 12):
                    proj_unit(rb, 0)
                    proj_unit(rb, 1)
                flush_pend()
                fq.drain()
                # keep-warm: the ~4.5us norm window below would otherwise
                # let HAM re-throttle and the final projs run at 1.2GHz;
                # cheap matmuls during the idle window keep K=8/8
                for w in range(10):
                    ps = fillp.tile([128, 512], F32, tag="fill",
                                    name="tailwarm")
                    nc.tensor.matmul(ps[:], warm[:, 0:128], warm[:],
                                     start=True, stop=True)
                # last block's norm: h0's reciprocal on DVE while h1 goes
                # through ACT (Ln then Exp(-x); two table loads, but ACT is
                # otherwise idle and this halves the serial DVE recip chain)
                u0 = norm_unit(0, 1, 1, 0)
                u0[0]()
                otu1 = otus.pop((0, 1, 1, 1))
                lnd = small.tile([1, 512], F32, tag="lnd", name="lnd")
                nc.scalar.activation(lnd[:], otu1[D:D + 1, :],
                                     mybir.ActivationFunctionType.Ln)
                rinv1 = small.tile([1, 512], F32, tag="rinva", name="rinva")
                nc.scalar.activation(rinv1[:], lnd[:], Exp, scale=-1.0)
                for u in u0[1:]:
                    u()
                rbig1 = small.tile([D, 512], F32, tag="rbiga", name="rbiga")
                nc.gpsimd.partition_broadcast(rbig1[:], rinv1[:])
                nc.vector.tensor_mul(
                    ot[(0, 1, 1)][D:2 * D, :], otu1[0:D, :], rbig1[:])
                for i, rb in enumerate(range(12, 16)):
                    proj_unit(rb, 0, eng="v" if i % 2 else "s")
                    proj_unit(rb, 1, eng="s" if i % 2 else "v")

    nc.compile()
    return nc


def get_nc():
    global _NC_CACHE
    if _NC_CACHE is None:
        _NC_CACHE = build_nc()
    return _NC_CACHE


def make_in_maps(x, w_qkv, w_proj):
    x = np.asarray(x, dtype=np.float32)
    w_qkv = np.asarray(w_qkv, dtype=np.float32)
    w_proj = np.asarray(w_proj, dtype=np.float32)
    xT = x.reshape(R, C).T.astype(np.float16)            # [C, R]
    # stage chunk-major/partition-contiguous: [rb, p, a, r] with
    # c = a*128 + p, t = rb*512 + r  ->  one contiguous read per chunk
    xTs = np.ascontiguousarray(
        xT.reshape(C // 128, 128, R // 512, 512).transpose(2, 1, 0, 3))
    in_maps = []
    for i in range(NCORES):
        h0, h1 = HPC * i, HPC * i + 1
        rows = []
        for part in range(3):  # q, k, v
            for h in (h0, h1):
                lo = part * C + h * D
                rows.append(w_qkv[lo:lo + D])
        w_slice = np.concatenate(rows, axis=0)           # [384, 1024]
        wT = w_slice.T.astype(np.float16)                # [1024, 384]
        wqkvT = np.ascontiguousarray(
            wT.reshape(C // 128, 128, 384).transpose(1, 0, 2))
        cols = np.r_[h0 * D:(h0 + 1) * D, h1 * D:(h1 + 1) * D]
        wprojT = np.ascontiguousarray(w_proj[:, cols].T.astype(np.float16))
        in_maps.append({"xT": xTs, "wqkvT": wqkvT, "wprojT": wprojT})
    return in_maps


def kernel(x, w_qkv, w_proj, b_proj):
    from concourse.bass_utils import run_bass_kernel_spmd

    nc = get_nc()
    in_maps = make_in_maps(x, w_qkv, w_proj)
    res = run_bass_kernel_spmd(nc, in_maps, core_ids=list(range(NCORES)))
    y = np.zeros((R, C), dtype=np.float32)
    for r in res.results:
        y += np.asarray(r["y"], dtype=np.float32)
    y += np.asarray(b_proj, dtype=np.float32)[None, :]
    return y.reshape(B, N, C)

